# revision 53
# baseline (speedup 1.0000x reference)
"""Trainium2 Bass kernel: batched tiny-window attention (B=6272, N=8, C=768, H=12).

Data-parallel over 8 NeuronCores (784 batches / 6272 tokens per core).
Design (~535us HW; PE-bound at ~97% mid-kernel occupancy, fp16 PE stream
floor for this algorithm is ~447us):
  - x pre-transposed + fp16-cast on HOST -> xT [C, TOK] DMA'd straight to SBUF
    (no on-chip x transposes/casts); output DMA'd fp16, upcast on host.
  - qkv/proj accumulation chains interleaved pairwise across PSUM banks
    (qkT 3 banks, v/proj 2) so weight loads hide under the paired chain's
    matmuls via the PE background weight buffer.
  - S = Q^T K runs as FULL K=128 matmuls (F=256 per head pair) against
    persistent BLOCK-DIAGONAL zero-padded k tiles (kbd): uniform full-array
    PE geometry avoids the 64-row config switches that expose weight loads
    (~105ns each) on every neighboring matmul.  k chunks are evacuated
    PSUM->kbd halves directly on VectorE; q chunks evacuated whole.
  - exp() batched 4 heads wide ([128,512] per ACT op, PSUM source); mask and
    rel-pos bias applied MULTIPLICATIVELY after exp (mask = exp(bias) *
    blockind, single [128, H*128] DMA): one wide DVE tensor_tensor + one
    wide tensor_reduce give masked row-sums; one reciprocal; per-head
    tensor_scalar normalize (DVE).  v evacuations ride ScalarE.
  - A is 8x8-block-diagonal, so the DVE 32x32 stream-transpose IS the exact
    transpose -> A^T in SBUF without PE transposes or PSUM round-trips.
  - MM2 pair-packed (tile_position col halves); all 6 pairs' outputs packed
    into 2 PSUM banks per group, evacuated with 2 batched ScalarE copies into
    one attnT tile that proj reads by column slice.
  - Two-stage macro software pipeline (qkv of macro m+1 emitted before
    attention/proj of macro m); the last two macros' CD phases interleave
    sub-by-sub to fill the pipeline tail.  qkv weights load first (split
    across two DMA queues); wproj/mask DMAs deferred past macro 0's loads.
Scale (hd^-0.5) and qkv bias are folded into host-side precomputed weights.
"""

import os
import sys
from contextlib import ExitStack

import numpy as np

# Recover automatically if a previous session left the NeuronCores wedged.
os.environ.setdefault("NEURON_RT_RESET_CORES", "1")

sys.path.insert(0, "/opt/trn_rl_repo")

import concourse.bass as bass  # noqa: E402
import concourse.bacc as bacc  # noqa: E402
import concourse.tile as tile  # noqa: E402
from concourse import mybir  # noqa: E402
from concourse.bass_utils import run_bass_kernel_spmd  # noqa: E402
from concourse.masks import make_identity  # noqa: E402

NCORES = 8
B, N, C = 6272, 8, 768
H, HD = 12, 64
OC = 3 * C
B_LOC = B // NCORES          # 784 batches per core
TOK = B_LOC * N              # 6272 tokens per core
CCH = C // 128               # 6 channel chunks
GRP = 128                    # tokens per attention group (16 batches)
MACRO = 512                  # tokens per macro tile
NPAIR = H // 2               # 6 head pairs

F16 = mybir.dt.float16
F32 = mybir.dt.float32

LAST_RESULT = {}             # test harness introspection (exec_time_ns etc.)


def _build_nc(use_bias: bool):
    nc = bacc.Bacc()
    x_ext = nc.declare_dram_parameter("xT", [C, TOK], F16, isOutput=False)
    wqkv_ext = nc.declare_dram_parameter("wqkvT", [C, OC], F16, isOutput=False)
    wproj_ext = nc.declare_dram_parameter("wprojT", [C, C], F16, isOutput=False)
    mk_ext = nc.declare_dram_parameter("mask", [GRP, H * GRP], F16, isOutput=False)
    if use_bias:
        qkb_ext = nc.declare_dram_parameter("qkb", [2 * C], F32, isOutput=False)
        vb_ext = nc.declare_dram_parameter("vb", [C], F32, isOutput=False)
    out_ext = nc.declare_dram_parameter("out", [TOK, C], F16, isOutput=True)

    macros = []
    t0 = 0
    while t0 < TOK:
        tw = min(MACRO, TOK - t0)
        macros.append((t0, tw))
        t0 += tw

    with tile.TileContext(nc) as tc, ExitStack() as ctx:
        wpool = ctx.enter_context(tc.tile_pool(name="weights", bufs=1))
        xTp = ctx.enter_context(tc.tile_pool(name="xT", bufs=18))
        qkTp = ctx.enter_context(tc.tile_pool(name="qkT", bufs=24))
        vp = ctx.enter_context(tc.tile_pool(name="v", bufs=12))
        attp = ctx.enter_context(tc.tile_pool(name="attnT", bufs=8))
        smallp = ctx.enter_context(tc.tile_pool(name="small", bufs=8))
        statp = ctx.enter_context(tc.tile_pool(name="stat", bufs=8))
        outp = ctx.enter_context(tc.tile_pool(name="outsb", bufs=4))
        # PSUM budget (8 banks): qkT accum 3, v/proj 2, S 2, op2 1
        ps_bqk = ctx.enter_context(tc.tile_pool(name="ps_bqk", bufs=3, space="PSUM"))
        ps_bvp = ctx.enter_context(tc.tile_pool(name="ps_bvp", bufs=2, space="PSUM"))
        ps_s = ctx.enter_context(tc.tile_pool(name="ps_s", bufs=2, space="PSUM"))
        ps_o = ctx.enter_context(tc.tile_pool(name="ps_o", bufs=1, space="PSUM"))

        # --- persistent weights / masks ---
        # qkv weights land first (one DMA per chunk) so macro 0's matmuls
        # can start ASAP; wproj/mask DMAs are deferred until after macro 0's
        # loads are enqueued (they're not needed until the first CD phase).
        wqkv = []
        for c in range(CCH):
            wt = wpool.tile([128, OC], F16, tag=f"wqkv{c}", name="wt")
            (nc.sync, nc.scalar)[c % 2].dma_start(
                out=wt, in_=wqkv_ext.ap()[c * 128:(c + 1) * 128, :])
            wqkv.append(wt)
        wproj = [wpool.tile([128, C], F16, tag=f"wproj{c}", name="wt")
                 for c in range(CCH)]
        maskall = wpool.tile([128, H * GRP], F16, tag="mask", name="maskall")

        def load_late_weights():
            for c in range(CCH):
                nc.sync.dma_start(
                    out=wproj[c], in_=wproj_ext.ap()[c * 128:(c + 1) * 128, :])
            nc.sync.dma_start(out=maskall, in_=mk_ext.ap())

        # Persistent block-diagonal k tiles (2 pipeline sets x 6 head pairs).
        # kbd[set][p][:, s*256:(s+1)*256] = [[k_{2p}, 0], [0, k_{2p+1}]] so
        # MM1 can run as full K=128 matmuls (uniform PE row config -> weight
        # loads pipeline through the background buffer; no 64/128-row mode
        # switches).  Zero halves are written once and never touched again.
        NSET = 2
        kbd = [[wpool.tile([128, 2 * MACRO], F16, tag=f"kbd{st}_{p}",
                           name=f"kbd{st}_{p}") for p in range(NPAIR)]
               for st in range(NSET)]
        # set 0 is needed by macro 0's k evacuations: zero it FAST (split
        # DVE/GpSimd); set 1 isn't needed until macro 1 -> GpSimd at leisure
        for p in range(NPAIR):
            eng = nc.vector if p % 2 == 0 else nc.gpsimd
            eng.memset(kbd[0][p][:, :], 0.0)
        for p in range(NPAIR):
            nc.gpsimd.memset(kbd[1][p][:, :], 0.0)

        qkb_t = vb_t = None
        if use_bias:
            qkb_t = wpool.tile([128, 2 * CCH], F32)
            nc.sync.dma_start(
                out=qkb_t, in_=qkb_ext.ap().rearrange("(a p) -> p a", p=128))
            vb_t = wpool.tile([128, C], F32)
            nc.sync.dma_start(out=vb_t, in_=vb_ext.ap().to_broadcast((128, C)))

        def emit_ab(t0, tw, st):
            """Phases A+B: xT load, qkv matmuls.  Returns state."""
            nsub = tw // GRP
            xT = [xTp.tile([128, MACRO], F16, tag="xt", name="xt")
                  for _ in range(CCH)]
            for c in range(CCH):
                # macro 0: spread DMA issue over the idle ACT/DVE HWDGE
                # queues so the serial ~600ns/issue Sync queue doesn't gate
                # the very first matmul chain
                eng = (nc.sync if t0 > 0 else
                       (nc.sync, nc.scalar)[c % 2])
                eng.dma_start(
                    out=xT[c][:, :tw],
                    in_=x_ext.ap()[c * 128:(c + 1) * 128, t0:t0 + tw])

            qkT = [qkTp.tile([128, MACRO], F16, tag="qkt", name="qkt")
                   for _ in range(CCH)]
            for jj in range(0, 2 * CCH, 2):
                # interleave two accumulation chains (separate PSUM banks) so
                # each chain's LDWEIGHTS hides under the other chain's matmul
                psq0 = ps_bqk.tile([128, 512], F32, tag="bqk", name="psq0")
                psq1 = ps_bqk.tile([128, 512], F32, tag="bqk", name="psq1")
                for c in range(CCH):
                    for k, psq in ((0, psq0), (1, psq1)):
                        nc.tensor.matmul(
                            psq[:, :tw],
                            lhsT=wqkv[c][:, (jj + k) * 128:(jj + k + 1) * 128],
                            rhs=xT[c][:, :tw],
                            start=(c == 0), stop=(c == CCH - 1))
                for k, psq in ((0, psq0), (1, psq1)):
                    j = jj + k
                    if j < CCH:
                        # q chunk: straight evacuation to SBUF
                        if use_bias:
                            nc.vector.tensor_scalar(
                                out=qkT[j][:, :tw], in0=psq[:, :tw],
                                scalar1=qkb_t[:, j:j + 1], scalar2=None,
                                op0=mybir.AluOpType.add)
                        else:
                            nc.vector.tensor_copy(out=qkT[j][:, :tw],
                                                  in_=psq[:, :tw])
                    else:
                        # k chunk: evacuate each 64-partition half straight
                        # into its diagonal block of the persistent kbd tile
                        # (off-diagonal halves stay zero).
                        p = j - CCH
                        for half in range(2):
                            psl_ = slice(64 * half, 64 * half + 64)
                            dst = kbd[st][p][psl_, 0:2 * tw].rearrange(
                                "p (s c) -> p s c", c=2 * GRP)[
                                :, :, half * GRP:(half + 1) * GRP]
                            src = psq[psl_, 0:tw].rearrange(
                                "p (s c) -> p s c", c=GRP)
                            if use_bias:
                                nc.vector.tensor_scalar(
                                    out=dst, in0=src,
                                    scalar1=qkb_t[psl_, j:j + 1], scalar2=None,
                                    op0=mybir.AluOpType.add)
                            else:
                                # ScalarE: keeps the DVE free for the
                                # softmax chain; ordering keeps these at the
                                # front of the ACT FIFO for this macro step
                                nc.scalar.copy(out=dst, in_=src)

            vt = [vp.tile([128, C], F16, tag="vt", name="vt")
                  for _ in range(nsub)]
            for s in range(nsub):
                psv0 = ps_bvp.tile([128, 512], F32, tag="bvp", name="psv0")
                psv1 = ps_bvp.tile([128, 512], F32, tag="bvp", name="psv1")
                for c in range(CCH):
                    for g, psv in ((0, psv0), (1, psv1)):
                        nc.tensor.matmul(
                            psv[:, 0:384],
                            lhsT=xT[c][:, s * GRP:(s + 1) * GRP],
                            rhs=wqkv[c][:, 2 * C + 384 * g:2 * C + 384 * (g + 1)],
                            start=(c == 0), stop=(c == CCH - 1))
                for g, psv in ((0, psv0), (1, psv1)):
                    if use_bias:
                        nc.vector.tensor_tensor(
                            out=vt[s][:, 384 * g:384 * (g + 1)],
                            in0=psv[:, 0:384],
                            in1=vb_t[:, 384 * g:384 * (g + 1)],
                            op=mybir.AluOpType.add)
                    else:
                        # v is latency-tolerant (consumed by MM2 a phase
                        # later) -> evacuate on ScalarE to unload the DVE
                        nc.scalar.copy(
                            out=vt[s][:, 384 * g:384 * (g + 1)],
                            in_=psv[:, 0:384])
            return (t0, tw, nsub, qkT, vt, st)

        def emit_cd_sub(state, s):
            """Phase C+D body for one 128-token sub-group of a macro."""
            t0, tw, nsub, qkT, vt, st = state
            if True:
                gsl = slice(s * GRP, (s + 1) * GRP)
                # --- S = Q^T K, one F=256 matmul per head PAIR against the
                # block-diag k tile (full K=128 row config, uniform with the
                # qkv/proj matmuls); exp batched 4 heads wide on ACT ---
                a_raw = smallp.tile([128, H * GRP], F16, tag="a", bufs=8,
                                    name="a_raw")
                for b in range(NPAIR // 2):
                    sq = ps_s.tile([128, 512], F32, tag="s", name="sq")
                    for k in range(2):
                        p = 2 * b + k
                        nc.tensor.matmul(
                            sq[:, k * 256:(k + 1) * 256],
                            lhsT=qkT[p][:, gsl],                  # q pair^T
                            rhs=kbd[st][p][:, s * 2 * GRP:(s + 1) * 2 * GRP],
                            start=True, stop=True)
                    nc.scalar.activation(
                        out=a_raw[:, b * 512:(b + 1) * 512], in_=sq,
                        func=mybir.ActivationFunctionType.Exp)
                a_net = smallp.tile([128, H * GRP], F16, tag="an", bufs=8,
                                    name="a_net")
                nc.vector.tensor_tensor(
                    out=a_net, in0=a_raw, in1=maskall,
                    op=mybir.AluOpType.mult)
                rs = statp.tile([128, H], F32, tag="rs", name="rs")
                nc.vector.tensor_reduce(
                    out=rs,
                    in_=a_net.rearrange("p (a b) -> p a b", a=H),
                    axis=mybir.AxisListType.X,
                    op=mybir.AluOpType.add)
                rc = statp.tile([128, H], F32, tag="rc", name="rc")
                nc.vector.reciprocal(out=rc, in_=rs)

                # --- normalize + transpose for ALL pairs first, then all 12
                # MM2s as one contiguous PE block: MM2 is the only non-full-
                # array config (M=64 col-tiled), and every adjacency with a
                # full-config matmul exposes a weight load (~105ns); batching
                # cuts transitions from ~28 to ~4 per sub ---
                op4a = ps_o.tile([128, 512], F32, tag="o", name="op4a")
                op4b = ps_o.tile([128, 512], F32, tag="o", name="op4b")
                at2s_l = []
                for p in range(NPAIR):
                    an = smallp.tile([128, 2 * GRP], F16, tag="anorm", name="an")
                    for half in range(2):
                        h = 2 * p + half
                        hsl = slice(half * GRP, (half + 1) * GRP)
                        nc.vector.tensor_scalar(
                            out=an[:, hsl],
                            in0=a_net[:, h * GRP:(h + 1) * GRP],
                            scalar1=rc[:, h:h + 1], scalar2=None,
                            op0=mybir.AluOpType.mult)
                    # A is 8x8-block-diagonal inside 32-aligned blocks, so a
                    # 32x32 block transpose IS the full transpose (off-diagonal
                    # blocks are exactly zero).
                    at2s = smallp.tile([128, 2 * GRP], F16, tag="at2s",
                                       bufs=12, name="at2s")
                    nc.vector.transpose(out=at2s, in_=an)
                    at2s_l.append(at2s)
                for p in range(NPAIR):
                    op4 = op4a if p < 4 else op4b
                    csl = slice(128 * (p % 4), 128 * (p % 4) + 128)
                    for half in range(2):
                        h = 2 * p + half
                        nc.tensor.matmul(
                            op4[64 * half:64 * (half + 1), csl],
                            lhsT=vt[s][:, h * 64:(h + 1) * 64],
                            rhs=at2s_l[p][:, half * GRP:(half + 1) * GRP],
                            start=True, stop=True,
                            tile_position=(0, 64 * half))
                attnT = attp.tile([128, CCH * GRP], F16, tag="att", name="attnT")
                nc.scalar.copy(out=attnT[:, 0:512], in_=op4a)
                nc.scalar.copy(out=attnT[:, 512:768], in_=op4b[:, 0:256])

                # --- Phase D: proj ---
                osb = outp.tile([128, C], F16, tag="osb")
                psp0 = ps_bvp.tile([128, 512], F32, tag="bvp", name="psp0")
                psp1 = ps_bvp.tile([128, 512], F32, tag="bvp", name="psp1")
                for c in range(CCH):
                    for g, psp in ((0, psp0), (1, psp1)):
                        nc.tensor.matmul(
                            psp[:, 0:384],
                            lhsT=attnT[:, c * 128:(c + 1) * 128],
                            rhs=wproj[c][:, 384 * g:384 * (g + 1)],
                            start=(c == 0), stop=(c == CCH - 1))
                for g, psp in ((0, psp0), (1, psp1)):
                    nc.scalar.copy(
                        out=osb[:, 384 * g:384 * (g + 1)], in_=psp[:, 0:384])
                nc.sync.dma_start(
                    out=out_ext.ap()[t0 + s * GRP: t0 + (s + 1) * GRP, :], in_=osb)

        def emit_cd(state):
            for s in range(state[2]):
                emit_cd_sub(state, s)

        # Two-stage software pipeline: macro m's attention/proj is emitted
        # after macro m+1's qkv, so the PE always has independent work.
        # The last three macros' CD phases are interleaved sub-by-sub so the
        # tail (which has no AB work left) has parallel chains to fill PE
        # gaps.
        TAILN = 2
        pendings = []
        for mi, (t0, tw) in enumerate(macros):
            st = emit_ab(t0, tw, mi % NSET)
            if mi == 0:
                load_late_weights()
            pendings.append((mi, st))
            if len(pendings) >= 2 and pendings[0][0] < len(macros) - TAILN:
                emit_cd(pendings.pop(0)[1])
        tail = [p_[1] for p_ in pendings]
        order = []
        for i in range(max(p_[2] for p_ in tail)):
            for p_ in tail:
                if i < p_[2]:
                    order.append((p_, i))
        for state_, s_ in order:
            emit_cd_sub(state_, s_)

    nc.compile()
    return nc


def make_host_inputs(x, qkv_w, qkv_b, proj_w, rel_bias_table):
    """Precompute device-side layouts (fp16, scale folded, x pre-transposed)."""
    scale = HD ** -0.5
    wq = qkv_w.copy()
    wq[:C] *= scale
    bq = qkv_b.copy()
    bq[:C] *= scale
    wqkvT = np.ascontiguousarray(wq.T).astype(np.float16)          # [C, 3C]
    wprojT = np.ascontiguousarray(proj_w.T).astype(np.float16)     # [C, C]

    # multiplicative mask per head: mask[h][i, m] = exp(bias(query=i, key=m))
    # on the block diagonal, 0 off-block.
    mk = np.zeros((H, GRP, GRP), np.float32)
    eb = np.exp(rel_bias_table)                                    # [15, H]
    for b in range(GRP // N):
        for i_ in range(N):      # query
            for m_ in range(N):  # key
                mk[:, b * N + i_, b * N + m_] = eb[m_ - i_ + N - 1, :]
    # device layout: [query row, h*128 + key] so it loads as ONE DMA
    mask = np.ascontiguousarray(
        mk.transpose(1, 0, 2).reshape(GRP, H * GRP)).astype(np.float16)

    x8 = x.reshape(NCORES, TOK, C)
    xT = np.ascontiguousarray(x8.transpose(0, 2, 1)).astype(np.float16)
    return xT, wqkvT, wprojT, mask, bq


_NC_CACHE = None


def kernel(x, qkv_w, qkv_b, proj_w, proj_b, rel_bias_table):
    global _NC_CACHE
    x = np.asarray(x, np.float32)
    qkv_w = np.asarray(qkv_w, np.float32)
    qkv_b = np.asarray(qkv_b, np.float32)
    proj_w = np.asarray(proj_w, np.float32)
    proj_b = np.asarray(proj_b, np.float32)
    tbl = np.asarray(rel_bias_table, np.float32)

    xT, wqkvT, wprojT, mask, bq = make_host_inputs(
        x, qkv_w, qkv_b, proj_w, tbl)

    use_bias = bool(np.any(qkv_b != 0))
    in_maps = []
    for i in range(NCORES):
        m = {"xT": xT[i], "wqkvT": wqkvT, "wprojT": wprojT, "mask": mask}
        if use_bias:
            m["qkb"] = np.ascontiguousarray(bq[:2 * C])
            m["vb"] = np.ascontiguousarray(qkv_b[2 * C:])
        in_maps.append(m)

    if _NC_CACHE is None or _NC_CACHE[0] != use_bias:
        _NC_CACHE = (use_bias, _build_nc(use_bias))
    nc = _NC_CACHE[1]

    trace = bool(int(os.environ.get("KERNEL_TRACE", "0")))
    res = run_bass_kernel_spmd(nc, in_maps, core_ids=list(range(NCORES)),
                               trace=trace)
    LAST_RESULT["exec_time_ns"] = getattr(res, "exec_time_ns", None)
    LAST_RESULT["res"] = res
    out = np.concatenate([np.asarray(r["out"]) for r in res.results], axis=0)
    out = out.reshape(B, N, C).astype(np.float32)
    out = out + proj_b[None, None, :]
    return out



# revision 58
# speedup vs baseline: 1.0141x; 1.0141x over previous
"""Trainium2 Bass kernel: batched tiny-window attention (B=6272, N=8, C=768, H=12).

Data-parallel over 8 NeuronCores (784 batches / 6272 tokens per core).
Design (~535us HW; PE-bound at ~97% mid-kernel occupancy, fp16 PE stream
floor for this algorithm is ~447us):
  - x pre-transposed + fp16-cast on HOST -> xT [C, TOK] DMA'd straight to SBUF
    (no on-chip x transposes/casts); output DMA'd fp16, upcast on host.
  - qkv/proj accumulation chains interleaved pairwise across PSUM banks
    (qkT 3 banks, v/proj 2) so weight loads hide under the paired chain's
    matmuls via the PE background weight buffer.
  - S = Q^T K runs as FULL K=128 matmuls (F=256 per head pair) against
    persistent BLOCK-DIAGONAL zero-padded k tiles (kbd): uniform full-array
    PE geometry avoids the 64-row config switches that expose weight loads
    (~105ns each) on every neighboring matmul.  k chunks are evacuated
    PSUM->kbd halves directly on VectorE; q chunks evacuated whole.
  - exp() batched 4 heads wide ([128,512] per ACT op, PSUM source); mask and
    rel-pos bias applied MULTIPLICATIVELY after exp (mask = exp(bias) *
    blockind, single [128, H*128] DMA): one wide DVE tensor_tensor + one
    wide tensor_reduce give masked row-sums; one reciprocal; per-head
    tensor_scalar normalize (DVE).  v evacuations ride ScalarE.
  - A is 8x8-block-diagonal, so the DVE 32x32 stream-transpose IS the exact
    transpose -> A^T in SBUF without PE transposes or PSUM round-trips.
  - MM2 pair-packed (tile_position col halves); all 6 pairs' outputs packed
    into 2 PSUM banks per group, evacuated with 2 batched ScalarE copies into
    one attnT tile that proj reads by column slice.
  - Two-stage macro software pipeline (qkv of macro m+1 emitted before
    attention/proj of macro m); the last two macros' CD phases interleave
    sub-by-sub to fill the pipeline tail.  qkv weights load first (split
    across two DMA queues); wproj/mask DMAs deferred past macro 0's loads.
Scale (hd^-0.5) and qkv bias are folded into host-side precomputed weights.
"""

import os
import sys
from contextlib import ExitStack

import numpy as np

# Recover automatically if a previous session left the NeuronCores wedged.
os.environ.setdefault("NEURON_RT_RESET_CORES", "1")

sys.path.insert(0, "/opt/trn_rl_repo")

import concourse.bass as bass  # noqa: E402
import concourse.bacc as bacc  # noqa: E402
import concourse.tile as tile  # noqa: E402
from concourse import mybir  # noqa: E402
from concourse.bass_utils import run_bass_kernel_spmd  # noqa: E402
from concourse.masks import make_identity  # noqa: E402

NCORES = 8
B, N, C = 6272, 8, 768
H, HD = 12, 64
OC = 3 * C
B_LOC = B // NCORES          # 784 batches per core
TOK = B_LOC * N              # 6272 tokens per core
CCH = C // 128               # 6 channel chunks
GRP = 128                    # tokens per attention group (16 batches)
MACRO = 512                  # tokens per macro tile
NPAIR = H // 2               # 6 head pairs

F16 = mybir.dt.float16
F32 = mybir.dt.float32

LAST_RESULT = {}             # test harness introspection (exec_time_ns etc.)


def _build_nc(use_bias: bool):
    nc = bacc.Bacc()
    x_ext = nc.declare_dram_parameter("xT", [C, TOK], F16, isOutput=False)
    wqkv_ext = nc.declare_dram_parameter("wqkvT", [C, OC], F16, isOutput=False)
    wproj_ext = nc.declare_dram_parameter("wprojT", [C, C], F16, isOutput=False)
    mk_ext = nc.declare_dram_parameter("mask", [GRP, H * GRP], F16, isOutput=False)
    if use_bias:
        qkb_ext = nc.declare_dram_parameter("qkb", [2 * C], F32, isOutput=False)
        vb_ext = nc.declare_dram_parameter("vb", [C], F32, isOutput=False)
    out_ext = nc.declare_dram_parameter("out", [TOK, C], F16, isOutput=True)

    macros = []
    t0 = 0
    while t0 < TOK:
        tw = min(MACRO, TOK - t0)
        macros.append((t0, tw))
        t0 += tw

    with tile.TileContext(nc) as tc, ExitStack() as ctx:
        wpool = ctx.enter_context(tc.tile_pool(name="weights", bufs=1))
        xTp = ctx.enter_context(tc.tile_pool(name="xT", bufs=18))
        qkTp = ctx.enter_context(tc.tile_pool(name="qkT", bufs=24))
        vp = ctx.enter_context(tc.tile_pool(name="v", bufs=8))
        attp = ctx.enter_context(tc.tile_pool(name="attnT", bufs=8))
        smallp = ctx.enter_context(tc.tile_pool(name="small", bufs=8))
        statp = ctx.enter_context(tc.tile_pool(name="stat", bufs=8))
        outp = ctx.enter_context(tc.tile_pool(name="outsb", bufs=4))
        # PSUM budget (8 banks): qkT accum 3, v/proj 2, S 2, op2 1
        ps_bqk = ctx.enter_context(tc.tile_pool(name="ps_bqk", bufs=3, space="PSUM"))
        ps_bvp = ctx.enter_context(tc.tile_pool(name="ps_bvp", bufs=2, space="PSUM"))
        ps_s = ctx.enter_context(tc.tile_pool(name="ps_s", bufs=2, space="PSUM"))
        ps_o = ctx.enter_context(tc.tile_pool(name="ps_o", bufs=1, space="PSUM"))

        # --- persistent weights / masks ---
        # qkv weights land first (one DMA per chunk) so macro 0's matmuls
        # can start ASAP; wproj/mask DMAs are deferred until after macro 0's
        # loads are enqueued (they're not needed until the first CD phase).
        wqkv = []
        for c in range(CCH):
            wt = wpool.tile([128, OC], F16, tag=f"wqkv{c}", name="wt")
            (nc.sync, nc.scalar)[c % 2].dma_start(
                out=wt, in_=wqkv_ext.ap()[c * 128:(c + 1) * 128, :])
            wqkv.append(wt)
        wproj = [wpool.tile([128, C], F16, tag=f"wproj{c}", name="wt")
                 for c in range(CCH)]
        maskall = wpool.tile([128, H * GRP], F16, tag="mask", name="maskall")

        def load_late_weights():
            for c in range(CCH):
                nc.sync.dma_start(
                    out=wproj[c], in_=wproj_ext.ap()[c * 128:(c + 1) * 128, :])
            nc.sync.dma_start(out=maskall, in_=mk_ext.ap())

        # Persistent block-diagonal k tiles (2 pipeline sets x 6 head pairs).
        # kbd[set][p][:, s*256:(s+1)*256] = [[k_{2p}, 0], [0, k_{2p+1}]] so
        # MM1 can run as full K=128 matmuls (uniform PE row config -> weight
        # loads pipeline through the background buffer; no 64/128-row mode
        # switches).  Zero halves are written once and never touched again.
        NSET = 2
        kbd = [[wpool.tile([128, 2 * MACRO], F16, tag=f"kbd{st}_{p}",
                           name=f"kbd{st}_{p}") for p in range(NPAIR)]
               for st in range(NSET)]
        # set 0 is needed by macro 0's k evacuations: zero it FAST (split
        # DVE/GpSimd); set 1 isn't needed until macro 1 -> GpSimd at leisure
        for p in range(NPAIR):
            eng = nc.vector if p % 2 == 0 else nc.gpsimd
            eng.memset(kbd[0][p][:, :], 0.0)
        for p in range(NPAIR):
            nc.gpsimd.memset(kbd[1][p][:, :], 0.0)

        # Zero-padded v tiles for full-array (M=128) MM2: vbd[set][s][g] is
        # [128 keys, 768] = three 256-col pair blocks [v_even | 0 | v_odd | 0]
        # (lhsT slice [256p : 256p+128] = [v|0], [256p+64 : 256p+192] =
        # [0|v]); the two MMs of a pair accumulate into the full 128-
        # partition output block, keeping MM2 on the same uniform PE config
        # as every other matmul.  vbd is filled from the vt staging tile by
        # the DMA engines (13% busy) so no compute-engine queue is touched.
        MSUB = MACRO // GRP
        vbd = [[[wpool.tile([128, 768], F16, tag=f"vbd{st}_{s}_{g}",
                            name=f"vbd{st}_{s}_{g}") for g in range(2)]
                for s in range(MSUB)]
               for st in range(NSET)]
        for st in range(NSET):
            for s in range(MSUB):
                for g in range(2):
                    eng = (nc.vector if (st == 0 and g == 0) else nc.gpsimd)
                    eng.memset(vbd[st][s][g][:, :], 0.0)

        qkb_t = vb_t = None
        if use_bias:
            qkb_t = wpool.tile([128, 2 * CCH], F32)
            nc.sync.dma_start(
                out=qkb_t, in_=qkb_ext.ap().rearrange("(a p) -> p a", p=128))
            vb_t = wpool.tile([128, C], F32)
            nc.sync.dma_start(out=vb_t, in_=vb_ext.ap().to_broadcast((128, C)))

        def emit_ab(t0, tw, st):
            """Phases A+B: xT load, qkv matmuls.  Returns state."""
            nsub = tw // GRP
            xT = [xTp.tile([128, MACRO], F16, tag="xt", name="xt")
                  for _ in range(CCH)]
            for c in range(CCH):
                # macro 0: spread DMA issue over the idle ACT/DVE HWDGE
                # queues so the serial ~600ns/issue Sync queue doesn't gate
                # the very first matmul chain
                eng = (nc.sync if t0 > 0 else
                       (nc.sync, nc.scalar)[c % 2])
                eng.dma_start(
                    out=xT[c][:, :tw],
                    in_=x_ext.ap()[c * 128:(c + 1) * 128, t0:t0 + tw])

            qkT = [qkTp.tile([128, MACRO], F16, tag="qkt", name="qkt")
                   for _ in range(CCH)]
            for jj in range(0, 2 * CCH, 2):
                # interleave two accumulation chains (separate PSUM banks) so
                # each chain's LDWEIGHTS hides under the other chain's matmul
                psq0 = ps_bqk.tile([128, 512], F32, tag="bqk", name="psq0")
                psq1 = ps_bqk.tile([128, 512], F32, tag="bqk", name="psq1")
                for c in range(CCH):
                    for k, psq in ((0, psq0), (1, psq1)):
                        nc.tensor.matmul(
                            psq[:, :tw],
                            lhsT=wqkv[c][:, (jj + k) * 128:(jj + k + 1) * 128],
                            rhs=xT[c][:, :tw],
                            start=(c == 0), stop=(c == CCH - 1))
                for k, psq in ((0, psq0), (1, psq1)):
                    j = jj + k
                    if j < CCH:
                        # q chunk: straight evacuation to SBUF
                        if use_bias:
                            nc.vector.tensor_scalar(
                                out=qkT[j][:, :tw], in0=psq[:, :tw],
                                scalar1=qkb_t[:, j:j + 1], scalar2=None,
                                op0=mybir.AluOpType.add)
                        else:
                            nc.vector.tensor_copy(out=qkT[j][:, :tw],
                                                  in_=psq[:, :tw])
                    else:
                        # k chunk: evacuate each 64-partition half straight
                        # into its diagonal block of the persistent kbd tile
                        # (off-diagonal halves stay zero).
                        p = j - CCH
                        for half in range(2):
                            psl_ = slice(64 * half, 64 * half + 64)
                            dst = kbd[st][p][psl_, 0:2 * tw].rearrange(
                                "p (s c) -> p s c", c=2 * GRP)[
                                :, :, half * GRP:(half + 1) * GRP]
                            src = psq[psl_, 0:tw].rearrange(
                                "p (s c) -> p s c", c=GRP)
                            if use_bias:
                                nc.vector.tensor_scalar(
                                    out=dst, in0=src,
                                    scalar1=qkb_t[psl_, j:j + 1], scalar2=None,
                                    op0=mybir.AluOpType.add)
                            else:
                                # ScalarE: keeps the DVE free for the
                                # softmax chain; ordering keeps these at the
                                # front of the ACT FIFO for this macro step
                                nc.scalar.copy(out=dst, in_=src)

            vt = [vp.tile([128, C], F16, tag="vt", name="vt")
                  for _ in range(nsub)]
            for s in range(nsub):
                psv0 = ps_bvp.tile([128, 512], F32, tag="bvp", name="psv0")
                psv1 = ps_bvp.tile([128, 512], F32, tag="bvp", name="psv1")
                for c in range(CCH):
                    for g, psv in ((0, psv0), (1, psv1)):
                        nc.tensor.matmul(
                            psv[:, 0:384],
                            lhsT=xT[c][:, s * GRP:(s + 1) * GRP],
                            rhs=wqkv[c][:, 2 * C + 384 * g:2 * C + 384 * (g + 1)],
                            start=(c == 0), stop=(c == CCH - 1))
                for g, psv in ((0, psv0), (1, psv1)):
                    if use_bias:
                        nc.vector.tensor_tensor(
                            out=vt[s][:, 384 * g:384 * (g + 1)],
                            in0=psv[:, 0:384],
                            in1=vb_t[:, 384 * g:384 * (g + 1)],
                            op=mybir.AluOpType.add)
                    else:
                        # v is latency-tolerant (consumed by MM2 a phase
                        # later) -> evacuate on ScalarE to unload the DVE
                        nc.scalar.copy(
                            out=vt[s][:, 384 * g:384 * (g + 1)],
                            in_=psv[:, 0:384])
                # spread staged v into the zero-padded pair-block layout on
                # the DMA engines (off every compute queue; consumed by MM2
                # a full phase later so the ~5us DMA latency is free)
                for g in range(2):
                    dst = vbd[st][s][g].rearrange(
                        "p (pr half b) -> p pr half b",
                        pr=3, half=2)[:, :, :, 0:64]
                    src = vt[s][:, 384 * g:384 * (g + 1)].rearrange(
                        "p (pr half b) -> p pr half b", pr=3, half=2)
                    (nc.sync, nc.scalar)[g].dma_start(out=dst, in_=src)
            return (t0, tw, nsub, qkT, vt, st)

        def emit_cd_sub(state, s):
            """Phase C+D body for one 128-token sub-group of a macro."""
            t0, tw, nsub, qkT, vt, st = state
            if True:
                gsl = slice(s * GRP, (s + 1) * GRP)
                # --- S = Q^T K, one F=256 matmul per head PAIR against the
                # block-diag k tile (full K=128 row config, uniform with the
                # qkv/proj matmuls); exp batched 4 heads wide on ACT ---
                a_raw = smallp.tile([128, H * GRP], F16, tag="a", bufs=6,
                                    name="a_raw")
                for b in range(NPAIR // 2):
                    sq = ps_s.tile([128, 512], F32, tag="s", name="sq")
                    for k in range(2):
                        p = 2 * b + k
                        nc.tensor.matmul(
                            sq[:, k * 256:(k + 1) * 256],
                            lhsT=qkT[p][:, gsl],                  # q pair^T
                            rhs=kbd[st][p][:, s * 2 * GRP:(s + 1) * 2 * GRP],
                            start=True, stop=True)
                    nc.scalar.activation(
                        out=a_raw[:, b * 512:(b + 1) * 512], in_=sq,
                        func=mybir.ActivationFunctionType.Exp)
                a_net = smallp.tile([128, H * GRP], F16, tag="an", bufs=6,
                                    name="a_net")
                nc.vector.tensor_tensor(
                    out=a_net, in0=a_raw, in1=maskall,
                    op=mybir.AluOpType.mult)
                rs = statp.tile([128, H], F32, tag="rs", name="rs")
                nc.vector.tensor_reduce(
                    out=rs,
                    in_=a_net.rearrange("p (a b) -> p a b", a=H),
                    axis=mybir.AxisListType.X,
                    op=mybir.AluOpType.add)
                rc = statp.tile([128, H], F32, tag="rc", name="rc")
                nc.vector.reciprocal(out=rc, in_=rs)

                # --- normalize + transpose for ALL pairs first, then all 12
                # MM2s as one contiguous PE block: MM2 is the only non-full-
                # array config (M=64 col-tiled), and every adjacency with a
                # full-config matmul exposes a weight load (~105ns); batching
                # cuts transitions from ~28 to ~4 per sub ---
                op4a = ps_o.tile([128, 512], F32, tag="o", name="op4a")
                op4b = ps_o.tile([128, 512], F32, tag="o", name="op4b")
                at2s_l = []
                for p in range(NPAIR):
                    an = smallp.tile([128, 2 * GRP], F16, tag="anorm", name="an")
                    for half in range(2):
                        h = 2 * p + half
                        hsl = slice(half * GRP, (half + 1) * GRP)
                        nc.vector.tensor_scalar(
                            out=an[:, hsl],
                            in0=a_net[:, h * GRP:(h + 1) * GRP],
                            scalar1=rc[:, h:h + 1], scalar2=None,
                            op0=mybir.AluOpType.mult)
                    # A is 8x8-block-diagonal inside 32-aligned blocks, so a
                    # 32x32 block transpose IS the full transpose (off-diagonal
                    # blocks are exactly zero).
                    at2s = smallp.tile([128, 2 * GRP], F16, tag="at2s",
                                       bufs=12, name="at2s")
                    nc.vector.transpose(out=at2s, in_=an)
                    at2s_l.append(at2s)
                for p in range(NPAIR):
                    op4 = op4a if p < 4 else op4b
                    csl = slice(128 * (p % 4), 128 * (p % 4) + 128)
                    g, pi = p // 3, p % 3
                    for half in range(2):
                        nc.tensor.matmul(
                            op4[:, csl],
                            lhsT=vbd[st][s][g][:, 256 * pi + 64 * half:
                                               256 * pi + 64 * half + 128],
                            rhs=at2s_l[p][:, half * GRP:(half + 1) * GRP],
                            start=(half == 0), stop=(half == 1))
                attnT = attp.tile([128, CCH * GRP], F16, tag="att", name="attnT")
                nc.scalar.copy(out=attnT[:, 0:512], in_=op4a)
                nc.scalar.copy(out=attnT[:, 512:768], in_=op4b[:, 0:256])

                # --- Phase D: proj ---
                osb = outp.tile([128, C], F16, tag="osb")
                psp0 = ps_bvp.tile([128, 512], F32, tag="bvp", name="psp0")
                psp1 = ps_bvp.tile([128, 512], F32, tag="bvp", name="psp1")
                for c in range(CCH):
                    for g, psp in ((0, psp0), (1, psp1)):
                        nc.tensor.matmul(
                            psp[:, 0:384],
                            lhsT=attnT[:, c * 128:(c + 1) * 128],
                            rhs=wproj[c][:, 384 * g:384 * (g + 1)],
                            start=(c == 0), stop=(c == CCH - 1))
                for g, psp in ((0, psp0), (1, psp1)):
                    nc.scalar.copy(
                        out=osb[:, 384 * g:384 * (g + 1)], in_=psp[:, 0:384])
                nc.sync.dma_start(
                    out=out_ext.ap()[t0 + s * GRP: t0 + (s + 1) * GRP, :], in_=osb)

        def emit_cd(state):
            for s in range(state[2]):
                emit_cd_sub(state, s)

        # Two-stage software pipeline: macro m's attention/proj is emitted
        # after macro m+1's qkv, so the PE always has independent work.
        # The last three macros' CD phases are interleaved sub-by-sub so the
        # tail (which has no AB work left) has parallel chains to fill PE
        # gaps.
        TAILN = 2
        pendings = []
        for mi, (t0, tw) in enumerate(macros):
            st = emit_ab(t0, tw, mi % NSET)
            if mi == 0:
                load_late_weights()
            pendings.append((mi, st))
            if len(pendings) >= 2 and pendings[0][0] < len(macros) - TAILN:
                emit_cd(pendings.pop(0)[1])
        tail = [p_[1] for p_ in pendings]
        order = []
        for i in range(max(p_[2] for p_ in tail)):
            for p_ in tail:
                if i < p_[2]:
                    order.append((p_, i))
        for state_, s_ in order:
            emit_cd_sub(state_, s_)

    nc.compile()
    return nc


def make_host_inputs(x, qkv_w, qkv_b, proj_w, rel_bias_table):
    """Precompute device-side layouts (fp16, scale folded, x pre-transposed)."""
    scale = HD ** -0.5
    wq = qkv_w.copy()
    wq[:C] *= scale
    bq = qkv_b.copy()
    bq[:C] *= scale
    wqkvT = np.ascontiguousarray(wq.T).astype(np.float16)          # [C, 3C]
    wprojT = np.ascontiguousarray(proj_w.T).astype(np.float16)     # [C, C]

    # multiplicative mask per head: mask[h][i, m] = exp(bias(query=i, key=m))
    # on the block diagonal, 0 off-block.
    mk = np.zeros((H, GRP, GRP), np.float32)
    eb = np.exp(rel_bias_table)                                    # [15, H]
    for b in range(GRP // N):
        for i_ in range(N):      # query
            for m_ in range(N):  # key
                mk[:, b * N + i_, b * N + m_] = eb[m_ - i_ + N - 1, :]
    # device layout: [query row, h*128 + key] so it loads as ONE DMA
    mask = np.ascontiguousarray(
        mk.transpose(1, 0, 2).reshape(GRP, H * GRP)).astype(np.float16)

    x8 = x.reshape(NCORES, TOK, C)
    xT = np.ascontiguousarray(x8.transpose(0, 2, 1)).astype(np.float16)
    return xT, wqkvT, wprojT, mask, bq


_NC_CACHE = None


def kernel(x, qkv_w, qkv_b, proj_w, proj_b, rel_bias_table):
    global _NC_CACHE
    x = np.asarray(x, np.float32)
    qkv_w = np.asarray(qkv_w, np.float32)
    qkv_b = np.asarray(qkv_b, np.float32)
    proj_w = np.asarray(proj_w, np.float32)
    proj_b = np.asarray(proj_b, np.float32)
    tbl = np.asarray(rel_bias_table, np.float32)

    xT, wqkvT, wprojT, mask, bq = make_host_inputs(
        x, qkv_w, qkv_b, proj_w, tbl)

    use_bias = bool(np.any(qkv_b != 0))
    in_maps = []
    for i in range(NCORES):
        m = {"xT": xT[i], "wqkvT": wqkvT, "wprojT": wprojT, "mask": mask}
        if use_bias:
            m["qkb"] = np.ascontiguousarray(bq[:2 * C])
            m["vb"] = np.ascontiguousarray(qkv_b[2 * C:])
        in_maps.append(m)

    if _NC_CACHE is None or _NC_CACHE[0] != use_bias:
        _NC_CACHE = (use_bias, _build_nc(use_bias))
    nc = _NC_CACHE[1]

    trace = bool(int(os.environ.get("KERNEL_TRACE", "0")))
    res = run_bass_kernel_spmd(nc, in_maps, core_ids=list(range(NCORES)),
                               trace=trace)
    LAST_RESULT["exec_time_ns"] = getattr(res, "exec_time_ns", None)
    LAST_RESULT["res"] = res
    out = np.concatenate([np.asarray(r["out"]) for r in res.results], axis=0)
    out = out.reshape(B, N, C).astype(np.float32)
    out = out + proj_b[None, None, :]
    return out



# revision 59
# speedup vs baseline: 1.0168x; 1.0027x over previous
"""Trainium2 Bass kernel: batched tiny-window attention (B=6272, N=8, C=768, H=12).

Data-parallel over 8 NeuronCores (784 batches / 6272 tokens per core).
Design (~535us HW; PE-bound at ~97% mid-kernel occupancy, fp16 PE stream
floor for this algorithm is ~447us):
  - x pre-transposed + fp16-cast on HOST -> xT [C, TOK] DMA'd straight to SBUF
    (no on-chip x transposes/casts); output DMA'd fp16, upcast on host.
  - qkv/proj accumulation chains interleaved pairwise across PSUM banks
    (qkT 3 banks, v/proj 2) so weight loads hide under the paired chain's
    matmuls via the PE background weight buffer.
  - S = Q^T K runs as FULL K=128 matmuls (F=256 per head pair) against
    persistent BLOCK-DIAGONAL zero-padded k tiles (kbd): uniform full-array
    PE geometry avoids the 64-row config switches that expose weight loads
    (~105ns each) on every neighboring matmul.  k chunks are evacuated
    PSUM->kbd halves directly on VectorE; q chunks evacuated whole.
  - exp() batched 4 heads wide ([128,512] per ACT op, PSUM source); mask and
    rel-pos bias applied MULTIPLICATIVELY after exp (mask = exp(bias) *
    blockind, single [128, H*128] DMA): one wide DVE tensor_tensor + one
    wide tensor_reduce give masked row-sums; one reciprocal; per-head
    tensor_scalar normalize (DVE).  v evacuations ride ScalarE.
  - A is 8x8-block-diagonal, so the DVE 32x32 stream-transpose IS the exact
    transpose -> A^T in SBUF without PE transposes or PSUM round-trips.
  - MM2 pair-packed (tile_position col halves); all 6 pairs' outputs packed
    into 2 PSUM banks per group, evacuated with 2 batched ScalarE copies into
    one attnT tile that proj reads by column slice.
  - Two-stage macro software pipeline (qkv of macro m+1 emitted before
    attention/proj of macro m); the last two macros' CD phases interleave
    sub-by-sub to fill the pipeline tail.  qkv weights load first (split
    across two DMA queues); wproj/mask DMAs deferred past macro 0's loads.
Scale (hd^-0.5) and qkv bias are folded into host-side precomputed weights.
"""

import os
import sys
from contextlib import ExitStack

import numpy as np

# Recover automatically if a previous session left the NeuronCores wedged.
os.environ.setdefault("NEURON_RT_RESET_CORES", "1")

sys.path.insert(0, "/opt/trn_rl_repo")

import concourse.bass as bass  # noqa: E402
import concourse.bacc as bacc  # noqa: E402
import concourse.tile as tile  # noqa: E402
from concourse import mybir  # noqa: E402
from concourse.bass_utils import run_bass_kernel_spmd  # noqa: E402
from concourse.masks import make_identity  # noqa: E402

NCORES = 8
B, N, C = 6272, 8, 768
H, HD = 12, 64
OC = 3 * C
B_LOC = B // NCORES          # 784 batches per core
TOK = B_LOC * N              # 6272 tokens per core
CCH = C // 128               # 6 channel chunks
GRP = 128                    # tokens per attention group (16 batches)
MACRO = 512                  # tokens per macro tile
NPAIR = H // 2               # 6 head pairs

F16 = mybir.dt.float16
F32 = mybir.dt.float32

LAST_RESULT = {}             # test harness introspection (exec_time_ns etc.)


def _build_nc(use_bias: bool):
    nc = bacc.Bacc()
    x_ext = nc.declare_dram_parameter("xT", [C, TOK], F16, isOutput=False)
    wqkv_ext = nc.declare_dram_parameter("wqkvT", [C, OC], F16, isOutput=False)
    wproj_ext = nc.declare_dram_parameter("wprojT", [C, C], F16, isOutput=False)
    mk_ext = nc.declare_dram_parameter("mask", [GRP, H * GRP], F16, isOutput=False)
    if use_bias:
        qkb_ext = nc.declare_dram_parameter("qkb", [2 * C], F32, isOutput=False)
        vb_ext = nc.declare_dram_parameter("vb", [C], F32, isOutput=False)
    out_ext = nc.declare_dram_parameter("out", [TOK, C], F16, isOutput=True)

    # Last 640 tokens split 256+256+128: the pipeline tail is DVE-chain
    # bound (~6.6us of serial softmax work per 128-token sub with no AB
    # work left to overlap), so fewer subs in the tail macros = shorter
    # drain.
    sizes = [MACRO] * ((TOK - 640) // MACRO) + [256, 256, 128]
    assert sum(sizes) == TOK
    macros = []
    t0 = 0
    for tw in sizes:
        macros.append((t0, tw))
        t0 += tw

    with tile.TileContext(nc) as tc, ExitStack() as ctx:
        wpool = ctx.enter_context(tc.tile_pool(name="weights", bufs=1))
        xTp = ctx.enter_context(tc.tile_pool(name="xT", bufs=18))
        qkTp = ctx.enter_context(tc.tile_pool(name="qkT", bufs=24))
        vp = ctx.enter_context(tc.tile_pool(name="v", bufs=8))
        attp = ctx.enter_context(tc.tile_pool(name="attnT", bufs=8))
        smallp = ctx.enter_context(tc.tile_pool(name="small", bufs=8))
        statp = ctx.enter_context(tc.tile_pool(name="stat", bufs=8))
        outp = ctx.enter_context(tc.tile_pool(name="outsb", bufs=4))
        # PSUM budget (8 banks): qkT accum 3, v/proj 2, S 2, op2 1
        ps_bqk = ctx.enter_context(tc.tile_pool(name="ps_bqk", bufs=3, space="PSUM"))
        ps_bvp = ctx.enter_context(tc.tile_pool(name="ps_bvp", bufs=2, space="PSUM"))
        ps_s = ctx.enter_context(tc.tile_pool(name="ps_s", bufs=2, space="PSUM"))
        ps_o = ctx.enter_context(tc.tile_pool(name="ps_o", bufs=1, space="PSUM"))

        # --- persistent weights / masks ---
        # qkv weights land first (one DMA per chunk) so macro 0's matmuls
        # can start ASAP; wproj/mask DMAs are deferred until after macro 0's
        # loads are enqueued (they're not needed until the first CD phase).
        wqkv = []
        for c in range(CCH):
            wt = wpool.tile([128, OC], F16, tag=f"wqkv{c}", name="wt")
            (nc.sync, nc.scalar)[c % 2].dma_start(
                out=wt, in_=wqkv_ext.ap()[c * 128:(c + 1) * 128, :])
            wqkv.append(wt)
        wproj = [wpool.tile([128, C], F16, tag=f"wproj{c}", name="wt")
                 for c in range(CCH)]
        maskall = wpool.tile([128, H * GRP], F16, tag="mask", name="maskall")

        def load_late_weights():
            for c in range(CCH):
                nc.sync.dma_start(
                    out=wproj[c], in_=wproj_ext.ap()[c * 128:(c + 1) * 128, :])
            nc.sync.dma_start(out=maskall, in_=mk_ext.ap())

        # Persistent block-diagonal k tiles (2 pipeline sets x 6 head pairs).
        # kbd[set][p][:, s*256:(s+1)*256] = [[k_{2p}, 0], [0, k_{2p+1}]] so
        # MM1 can run as full K=128 matmuls (uniform PE row config -> weight
        # loads pipeline through the background buffer; no 64/128-row mode
        # switches).  Zero halves are written once and never touched again.
        NSET = 2
        kbd = [[wpool.tile([128, 2 * MACRO], F16, tag=f"kbd{st}_{p}",
                           name=f"kbd{st}_{p}") for p in range(NPAIR)]
               for st in range(NSET)]
        # set 0 is needed by macro 0's k evacuations: zero it FAST (split
        # DVE/GpSimd); set 1 isn't needed until macro 1 -> GpSimd at leisure
        for p in range(NPAIR):
            eng = nc.vector if p % 2 == 0 else nc.gpsimd
            eng.memset(kbd[0][p][:, :], 0.0)
        for p in range(NPAIR):
            nc.gpsimd.memset(kbd[1][p][:, :], 0.0)

        # Zero-padded v tiles for full-array (M=128) MM2: vbd[set][s][g] is
        # [128 keys, 768] = three 256-col pair blocks [v_even | 0 | v_odd | 0]
        # (lhsT slice [256p : 256p+128] = [v|0], [256p+64 : 256p+192] =
        # [0|v]); the two MMs of a pair accumulate into the full 128-
        # partition output block, keeping MM2 on the same uniform PE config
        # as every other matmul.  vbd is filled from the vt staging tile by
        # the DMA engines (13% busy) so no compute-engine queue is touched.
        MSUB = MACRO // GRP
        vbd = [[[wpool.tile([128, 768], F16, tag=f"vbd{st}_{s}_{g}",
                            name=f"vbd{st}_{s}_{g}") for g in range(2)]
                for s in range(MSUB)]
               for st in range(NSET)]
        for st in range(NSET):
            for s in range(MSUB):
                for g in range(2):
                    eng = (nc.vector if (st == 0 and g == 0) else nc.gpsimd)
                    eng.memset(vbd[st][s][g][:, :], 0.0)

        qkb_t = vb_t = None
        if use_bias:
            qkb_t = wpool.tile([128, 2 * CCH], F32)
            nc.sync.dma_start(
                out=qkb_t, in_=qkb_ext.ap().rearrange("(a p) -> p a", p=128))
            vb_t = wpool.tile([128, C], F32)
            nc.sync.dma_start(out=vb_t, in_=vb_ext.ap().to_broadcast((128, C)))

        def emit_ab(t0, tw, st):
            """Phases A+B: xT load, qkv matmuls.  Returns state."""
            nsub = tw // GRP
            xT = [xTp.tile([128, MACRO], F16, tag="xt", name="xt")
                  for _ in range(CCH)]
            for c in range(CCH):
                # macro 0: spread DMA issue over the idle ACT/DVE HWDGE
                # queues so the serial ~600ns/issue Sync queue doesn't gate
                # the very first matmul chain
                eng = (nc.sync if t0 > 0 else
                       (nc.sync, nc.scalar)[c % 2])
                eng.dma_start(
                    out=xT[c][:, :tw],
                    in_=x_ext.ap()[c * 128:(c + 1) * 128, t0:t0 + tw])

            qkT = [qkTp.tile([128, MACRO], F16, tag="qkt", name="qkt")
                   for _ in range(CCH)]
            for jj in range(0, 2 * CCH, 2):
                # interleave two accumulation chains (separate PSUM banks) so
                # each chain's LDWEIGHTS hides under the other chain's matmul
                psq0 = ps_bqk.tile([128, 512], F32, tag="bqk", name="psq0")
                psq1 = ps_bqk.tile([128, 512], F32, tag="bqk", name="psq1")
                for c in range(CCH):
                    for k, psq in ((0, psq0), (1, psq1)):
                        nc.tensor.matmul(
                            psq[:, :tw],
                            lhsT=wqkv[c][:, (jj + k) * 128:(jj + k + 1) * 128],
                            rhs=xT[c][:, :tw],
                            start=(c == 0), stop=(c == CCH - 1))
                for k, psq in ((0, psq0), (1, psq1)):
                    j = jj + k
                    if j < CCH:
                        # q chunk: straight evacuation to SBUF
                        if use_bias:
                            nc.vector.tensor_scalar(
                                out=qkT[j][:, :tw], in0=psq[:, :tw],
                                scalar1=qkb_t[:, j:j + 1], scalar2=None,
                                op0=mybir.AluOpType.add)
                        else:
                            nc.vector.tensor_copy(out=qkT[j][:, :tw],
                                                  in_=psq[:, :tw])
                    else:
                        # k chunk: evacuate each 64-partition half straight
                        # into its diagonal block of the persistent kbd tile
                        # (off-diagonal halves stay zero).
                        p = j - CCH
                        for half in range(2):
                            psl_ = slice(64 * half, 64 * half + 64)
                            dst = kbd[st][p][psl_, 0:2 * tw].rearrange(
                                "p (s c) -> p s c", c=2 * GRP)[
                                :, :, half * GRP:(half + 1) * GRP]
                            src = psq[psl_, 0:tw].rearrange(
                                "p (s c) -> p s c", c=GRP)
                            if use_bias:
                                nc.vector.tensor_scalar(
                                    out=dst, in0=src,
                                    scalar1=qkb_t[psl_, j:j + 1], scalar2=None,
                                    op0=mybir.AluOpType.add)
                            else:
                                # ScalarE: keeps the DVE free for the
                                # softmax chain; ordering keeps these at the
                                # front of the ACT FIFO for this macro step
                                nc.scalar.copy(out=dst, in_=src)

            vt = [vp.tile([128, C], F16, tag="vt", name="vt")
                  for _ in range(nsub)]
            for s in range(nsub):
                psv0 = ps_bvp.tile([128, 512], F32, tag="bvp", name="psv0")
                psv1 = ps_bvp.tile([128, 512], F32, tag="bvp", name="psv1")
                for c in range(CCH):
                    for g, psv in ((0, psv0), (1, psv1)):
                        nc.tensor.matmul(
                            psv[:, 0:384],
                            lhsT=xT[c][:, s * GRP:(s + 1) * GRP],
                            rhs=wqkv[c][:, 2 * C + 384 * g:2 * C + 384 * (g + 1)],
                            start=(c == 0), stop=(c == CCH - 1))
                for g, psv in ((0, psv0), (1, psv1)):
                    if use_bias:
                        nc.vector.tensor_tensor(
                            out=vt[s][:, 384 * g:384 * (g + 1)],
                            in0=psv[:, 0:384],
                            in1=vb_t[:, 384 * g:384 * (g + 1)],
                            op=mybir.AluOpType.add)
                    else:
                        # v is latency-tolerant (consumed by MM2 a phase
                        # later) -> evacuate on ScalarE to unload the DVE
                        nc.scalar.copy(
                            out=vt[s][:, 384 * g:384 * (g + 1)],
                            in_=psv[:, 0:384])
                # spread staged v into the zero-padded pair-block layout on
                # the DMA engines (off every compute queue; consumed by MM2
                # a full phase later so the ~5us DMA latency is free)
                for g in range(2):
                    dst = vbd[st][s][g].rearrange(
                        "p (pr half b) -> p pr half b",
                        pr=3, half=2)[:, :, :, 0:64]
                    src = vt[s][:, 384 * g:384 * (g + 1)].rearrange(
                        "p (pr half b) -> p pr half b", pr=3, half=2)
                    (nc.sync, nc.scalar)[g].dma_start(out=dst, in_=src)
            return (t0, tw, nsub, qkT, vt, st)

        def emit_cd_sub(state, s):
            """Phase C+D body for one 128-token sub-group of a macro."""
            t0, tw, nsub, qkT, vt, st = state
            if True:
                gsl = slice(s * GRP, (s + 1) * GRP)
                # --- S = Q^T K, one F=256 matmul per head PAIR against the
                # block-diag k tile (full K=128 row config, uniform with the
                # qkv/proj matmuls); exp batched 4 heads wide on ACT ---
                a_raw = smallp.tile([128, H * GRP], F16, tag="a", bufs=6,
                                    name="a_raw")
                for b in range(NPAIR // 2):
                    sq = ps_s.tile([128, 512], F32, tag="s", name="sq")
                    for k in range(2):
                        p = 2 * b + k
                        nc.tensor.matmul(
                            sq[:, k * 256:(k + 1) * 256],
                            lhsT=qkT[p][:, gsl],                  # q pair^T
                            rhs=kbd[st][p][:, s * 2 * GRP:(s + 1) * 2 * GRP],
                            start=True, stop=True)
                    nc.scalar.activation(
                        out=a_raw[:, b * 512:(b + 1) * 512], in_=sq,
                        func=mybir.ActivationFunctionType.Exp)
                a_net = smallp.tile([128, H * GRP], F16, tag="an", bufs=6,
                                    name="a_net")
                nc.vector.tensor_tensor(
                    out=a_net, in0=a_raw, in1=maskall,
                    op=mybir.AluOpType.mult)
                rs = statp.tile([128, H], F32, tag="rs", name="rs")
                nc.vector.tensor_reduce(
                    out=rs,
                    in_=a_net.rearrange("p (a b) -> p a b", a=H),
                    axis=mybir.AxisListType.X,
                    op=mybir.AluOpType.add)
                rc = statp.tile([128, H], F32, tag="rc", name="rc")
                nc.vector.reciprocal(out=rc, in_=rs)

                # --- normalize + transpose for ALL pairs first, then all 12
                # MM2s as one contiguous PE block: MM2 is the only non-full-
                # array config (M=64 col-tiled), and every adjacency with a
                # full-config matmul exposes a weight load (~105ns); batching
                # cuts transitions from ~28 to ~4 per sub ---
                op4a = ps_o.tile([128, 512], F32, tag="o", name="op4a")
                op4b = ps_o.tile([128, 512], F32, tag="o", name="op4b")
                at2s_l = []
                for p in range(NPAIR):
                    an = smallp.tile([128, 2 * GRP], F16, tag="anorm", name="an")
                    for half in range(2):
                        h = 2 * p + half
                        hsl = slice(half * GRP, (half + 1) * GRP)
                        nc.vector.tensor_scalar(
                            out=an[:, hsl],
                            in0=a_net[:, h * GRP:(h + 1) * GRP],
                            scalar1=rc[:, h:h + 1], scalar2=None,
                            op0=mybir.AluOpType.mult)
                    # A is 8x8-block-diagonal inside 32-aligned blocks, so a
                    # 32x32 block transpose IS the full transpose (off-diagonal
                    # blocks are exactly zero).
                    at2s = smallp.tile([128, 2 * GRP], F16, tag="at2s",
                                       bufs=12, name="at2s")
                    nc.vector.transpose(out=at2s, in_=an)
                    at2s_l.append(at2s)
                for p in range(NPAIR):
                    op4 = op4a if p < 4 else op4b
                    csl = slice(128 * (p % 4), 128 * (p % 4) + 128)
                    g, pi = p // 3, p % 3
                    for half in range(2):
                        nc.tensor.matmul(
                            op4[:, csl],
                            lhsT=vbd[st][s][g][:, 256 * pi + 64 * half:
                                               256 * pi + 64 * half + 128],
                            rhs=at2s_l[p][:, half * GRP:(half + 1) * GRP],
                            start=(half == 0), stop=(half == 1))
                attnT = attp.tile([128, CCH * GRP], F16, tag="att", name="attnT")
                nc.scalar.copy(out=attnT[:, 0:512], in_=op4a)
                nc.scalar.copy(out=attnT[:, 512:768], in_=op4b[:, 0:256])

                # --- Phase D: proj ---
                osb = outp.tile([128, C], F16, tag="osb")
                psp0 = ps_bvp.tile([128, 512], F32, tag="bvp", name="psp0")
                psp1 = ps_bvp.tile([128, 512], F32, tag="bvp", name="psp1")
                for c in range(CCH):
                    for g, psp in ((0, psp0), (1, psp1)):
                        nc.tensor.matmul(
                            psp[:, 0:384],
                            lhsT=attnT[:, c * 128:(c + 1) * 128],
                            rhs=wproj[c][:, 384 * g:384 * (g + 1)],
                            start=(c == 0), stop=(c == CCH - 1))
                for g, psp in ((0, psp0), (1, psp1)):
                    nc.scalar.copy(
                        out=osb[:, 384 * g:384 * (g + 1)], in_=psp[:, 0:384])
                nc.sync.dma_start(
                    out=out_ext.ap()[t0 + s * GRP: t0 + (s + 1) * GRP, :], in_=osb)

        def emit_cd(state):
            for s in range(state[2]):
                emit_cd_sub(state, s)

        # Two-stage software pipeline: macro m's attention/proj is emitted
        # after macro m+1's qkv, so the PE always has independent work.
        # The last three macros' CD phases are interleaved sub-by-sub so the
        # tail (which has no AB work left) has parallel chains to fill PE
        # gaps.
        TAILN = 2
        pendings = []
        for mi, (t0, tw) in enumerate(macros):
            st = emit_ab(t0, tw, mi % NSET)
            if mi == 0:
                load_late_weights()
            pendings.append((mi, st))
            if len(pendings) >= 2 and pendings[0][0] < len(macros) - TAILN:
                emit_cd(pendings.pop(0)[1])
        tail = [p_[1] for p_ in pendings]
        order = []
        for i in range(max(p_[2] for p_ in tail)):
            for p_ in tail:
                if i < p_[2]:
                    order.append((p_, i))
        for state_, s_ in order:
            emit_cd_sub(state_, s_)

    nc.compile()
    return nc


def make_host_inputs(x, qkv_w, qkv_b, proj_w, rel_bias_table):
    """Precompute device-side layouts (fp16, scale folded, x pre-transposed)."""
    scale = HD ** -0.5
    wq = qkv_w.copy()
    wq[:C] *= scale
    bq = qkv_b.copy()
    bq[:C] *= scale
    wqkvT = np.ascontiguousarray(wq.T).astype(np.float16)          # [C, 3C]
    wprojT = np.ascontiguousarray(proj_w.T).astype(np.float16)     # [C, C]

    # multiplicative mask per head: mask[h][i, m] = exp(bias(query=i, key=m))
    # on the block diagonal, 0 off-block.
    mk = np.zeros((H, GRP, GRP), np.float32)
    eb = np.exp(rel_bias_table)                                    # [15, H]
    for b in range(GRP // N):
        for i_ in range(N):      # query
            for m_ in range(N):  # key
                mk[:, b * N + i_, b * N + m_] = eb[m_ - i_ + N - 1, :]
    # device layout: [query row, h*128 + key] so it loads as ONE DMA
    mask = np.ascontiguousarray(
        mk.transpose(1, 0, 2).reshape(GRP, H * GRP)).astype(np.float16)

    x8 = x.reshape(NCORES, TOK, C)
    xT = np.ascontiguousarray(x8.transpose(0, 2, 1)).astype(np.float16)
    return xT, wqkvT, wprojT, mask, bq


_NC_CACHE = None


def kernel(x, qkv_w, qkv_b, proj_w, proj_b, rel_bias_table):
    global _NC_CACHE
    x = np.asarray(x, np.float32)
    qkv_w = np.asarray(qkv_w, np.float32)
    qkv_b = np.asarray(qkv_b, np.float32)
    proj_w = np.asarray(proj_w, np.float32)
    proj_b = np.asarray(proj_b, np.float32)
    tbl = np.asarray(rel_bias_table, np.float32)

    xT, wqkvT, wprojT, mask, bq = make_host_inputs(
        x, qkv_w, qkv_b, proj_w, tbl)

    use_bias = bool(np.any(qkv_b != 0))
    in_maps = []
    for i in range(NCORES):
        m = {"xT": xT[i], "wqkvT": wqkvT, "wprojT": wprojT, "mask": mask}
        if use_bias:
            m["qkb"] = np.ascontiguousarray(bq[:2 * C])
            m["vb"] = np.ascontiguousarray(qkv_b[2 * C:])
        in_maps.append(m)

    if _NC_CACHE is None or _NC_CACHE[0] != use_bias:
        _NC_CACHE = (use_bias, _build_nc(use_bias))
    nc = _NC_CACHE[1]

    trace = bool(int(os.environ.get("KERNEL_TRACE", "0")))
    res = run_bass_kernel_spmd(nc, in_maps, core_ids=list(range(NCORES)),
                               trace=trace)
    LAST_RESULT["exec_time_ns"] = getattr(res, "exec_time_ns", None)
    LAST_RESULT["res"] = res
    out = np.concatenate([np.asarray(r["out"]) for r in res.results], axis=0)
    out = out.reshape(B, N, C).astype(np.float32)
    out = out + proj_b[None, None, :]
    return out

